# revision 29
# baseline (speedup 1.0000x reference)
"""Masked (expander) linear layer on 8 Trainium2 NeuronCores.

Computes out = x @ (W * M)^T for
  x: [16384, 2048] f32, W: [2048, 2048] f32, M: [2048, 2048] int32 (0/1)

Sharding: pure data-parallel over rows of x. Each of the 8 cores gets 2048
rows of x plus a replicated copy of W and M, computes its [2048, 2048]
output shard entirely locally, and the host concatenates shards. No
collectives.

Device-side design (v6):
 - x and W are repacked to bf16 on the host (the same rounding a
   device-side cast-DMA would apply; 2.1e-3 rel err, far under
   tolerance), the mask to int8. Input HBM traffic per core is 20MB.
   The mask multiply (the module's elementwise FLOPs) runs on DVE; all
   matmul FLOPs run on PE.
 - Host layouts are PARTITION-MAJOR panels (x: [P, NB, KT, 512],
   W/mask: [NT, P, KT, 512]) so every DMA piece is 4-16KB contiguous
   per partition: descriptor generation is cheap and the stream runs at
   near line rate. This is what lets the PE start ~10us in without
   outrunning the stream.
 - Everything is SBUF-resident (wm 64KB/partition, x 64KB/partition,
   bf16); x is loaded exactly once.
 - All input DMAs ride the sync HWDGE ring in exact PE consumption
   order: x block0 (first quarter per single ktile) interleaved with
   n-chunk 0 of mask/W, then n-chunks 1-3, then x blocks 1-3. y
   evacuations ride the scalar ring (ACT copy + ACT-triggered DMA).
 - PE order: pairs (nt, b=0) for nt 0..3 first (paced by the W stream),
   then nt-outer over blocks 1..3 (everything resident by then). Steady
   state is LDWEIGHTS-hidden (bf16 fast-weight-load) and runs at the
   216ns/matmul streaming floor, gapless.
 - A warm-up burst of tiny matmuls on a scratch PSUM bank bridges the
   first DMA latency so the HAM clock-gate is at full rate (2.4 GHz)
   when real matmuls start.
 - PSUM groups rotate over all 8 banks; each group is evacuated right
   after it closes.
"""

from contextlib import ExitStack

import numpy as np
import ml_dtypes

import concourse.bacc as bacc
import concourse.bass as bass
import concourse.mybir as mybir
import concourse.tile as tile
from concourse.bass_utils import run_bass_kernel_spmd

N_CORES = 8
P = 128

FULL_N, FULL_OUT, FULL_IN = 16384, 2048, 2048


def build_nc(
    rows: int = FULL_N // N_CORES,
    in_dim: int = FULL_IN,
    out_dim: int = FULL_OUT,
    n_chunk: int = 512,
    m_block: int = 4,
    warmup_mms: int = 220,
):
    """Per-core Bass module: y[rows, out] = x @ (wt * m), bf16 inputs.

    DRAM layouts (partition-major panels):
      x  [P, NB, KT, m_block*P] bf16
      wt [NT, P, KT, n_chunk]   bf16
      mk [NT, P, KT, n_chunk]   int8
      y  [rows, out_dim]        f32 (row-major)
    """
    assert rows % P == 0 and in_dim % P == 0 and out_dim % n_chunk == 0
    KT = in_dim // P
    MT = rows // P
    NT = out_dim // n_chunk
    assert KT % 4 == 0 and MT % m_block == 0
    KQ = KT // 4
    NB = MT // m_block
    mw = m_block * P  # columns of x per block

    bf16 = mybir.dt.bfloat16

    KT2 = KT // 2  # mask bytes hold two adjacent ktiles (nibble packing)
    KQ2 = KQ // 2

    nc = bacc.Bacc("TRN2", target_bir_lowering=False, debug=False)
    x = nc.dram_tensor("x", [P, NB, KT, mw], bf16, kind="ExternalInput")
    wt = nc.dram_tensor("wt", [NT, P, KT, n_chunk], bf16, kind="ExternalInput")
    mk = nc.dram_tensor("mk", [NT, P, KT2, n_chunk], mybir.dt.int8, kind="ExternalInput")
    y = nc.dram_tensor("y", [rows, out_dim], mybir.dt.float32, kind="ExternalOutput")

    with ExitStack() as ctx:
        tc = ctx.enter_context(tile.TileContext(nc))
        wm_pool = ctx.enter_context(tc.tile_pool(name="wm", bufs=1))
        xt_pool = ctx.enter_context(tc.tile_pool(name="xt", bufs=1))
        ws_pool = ctx.enter_context(tc.tile_pool(name="ws", bufs=4))
        msk_pool = ctx.enter_context(tc.tile_pool(name="msk", bufs=4))
        yo_pool = ctx.enter_context(tc.tile_pool(name="yo", bufs=3))
        wu_pool = ctx.enter_context(tc.tile_pool(name="wu", bufs=1))
        pm_pool = ctx.enter_context(tc.tile_pool(name="pm", bufs=1, space="PSUM"))

        # Resident masked weight: wm_t[nt][q] of shape [P, KQ, n_chunk] bf16
        wm_t = [
            [
                wm_pool.tile([P, KQ, n_chunk], bf16, tag=f"wm{nt}_{q}", name=f"wm{nt}_{q}")
                for q in range(4)
            ]
            for nt in range(NT)
        ]
        # Resident x: xt_t[b][q] tiles [P, KQ, mw]
        xt_t = [
            [
                xt_pool.tile([P, KQ, mw], bf16, tag=f"xt{b}_{q}", name=f"xt{b}_{q}")
                for q in range(4)
            ]
            for b in range(NB)
        ]

        def x_ap(b, q, k):
            return xt_t[b][q][:, k, :]

        def wm_ap(nt, q, k):
            return wm_t[nt][q][:, k, :]

        # ---- PE warm-up: tiny matmuls on scratch data bridge the first
        # DMA latency and keep the HAM activity window busy so real
        # matmuls start at the full 2.4 GHz clock.
        if warmup_mms:
            wu = wu_pool.tile([P, P], bf16, tag="wu", name="wu")
            nc.vector.memset(wu[:], 0.0)
            pwu = pm_pool.tile([P, 64], mybir.dt.float32, tag="pm7", name="pmwu")
            for i in range(warmup_mms):
                nc.tensor.matmul(pwu[:], wu[:], wu[:, :64], start=True, stop=True)

        def load_x_piece(b, q):
            ksl = slice(q * KQ, (q + 1) * KQ)
            nc.sync.dma_start(out=xt_t[b][q][:], in_=x[:, b, ksl, :])

        def load_w_piece(nt, q):
            ksl = slice(q * KQ, (q + 1) * KQ)
            # mask bytes hold 2 ktiles: bit0 = even kt, bit1 = odd kt —
            # halves mask HBM bytes (the early stream is at line rate and
            # is what gates the PE start)
            mtile = msk_pool.tile([P, KQ2, n_chunk], mybir.dt.int8, tag="mt")
            nc.sync.dma_start(out=mtile[:], in_=mk[nt, :, q * KQ2 : (q + 1) * KQ2, :])
            wstage = ws_pool.tile([P, KQ, n_chunk], bf16, tag="ws")
            nc.sync.dma_start(out=wstage[:], in_=wt[nt, :, ksl, :])
            ma = msk_pool.tile([P, KQ2, n_chunk], mybir.dt.int8, tag="ma")
            mb_ = msk_pool.tile([P, KQ2, n_chunk], mybir.dt.int8, tag="mb")
            nc.vector.tensor_scalar(
                ma[:], mtile[:], 1, None, op0=mybir.AluOpType.bitwise_and
            )
            nc.vector.tensor_scalar(
                mb_[:], mtile[:], 1, None, op0=mybir.AluOpType.logical_shift_right
            )
            for k in range(KQ):
                src = ma if k % 2 == 0 else mb_
                nc.vector.tensor_mul(
                    wm_t[nt][q][:, k, :], wstage[:, k, :], src[:, k // 2, :]
                )

        # ---- input stream, in exact PE consumption order, all on the
        # sync HWDGE ring (FIFO): x0/nt0 interleaved, nt1-3, x1-3.
        for q in range(4):
            load_x_piece(0, q)
            load_w_piece(0, q)
        for nt in range(1, NT):
            for q in range(4):
                load_w_piece(nt, q)
        for b in range(1, NB):
            for q in range(4):
                load_x_piece(b, q)

        # ---- PE: pair (nt, b) = m_block PSUM groups of KT matmuls each
        pair_idx = [0]
        n_pairs = NT * NB

        def evac(nt, mt, psum_ap, csl=slice(0, None)):
            cols = (csl.stop or n_chunk) - (csl.start or 0)
            yo = yo_pool.tile([P, n_chunk], mybir.dt.float32, tag="yo")
            nc.scalar.copy(yo[:, 0:cols], psum_ap)
            nc.scalar.dma_start(
                out=y[
                    mt * P : (mt + 1) * P,
                    nt * n_chunk + (csl.start or 0) : nt * n_chunk
                    + (csl.start or 0)
                    + cols,
                ],
                in_=yo[:, 0:cols],
            )

        def pair(nt, b):
            g0 = pair_idx[0] * m_block
            last_pair = pair_idx[0] == n_pairs - 1
            pair_idx[0] += 1
            last_mb = m_block - 1 if last_pair else None
            pms = {
                mb: pm_pool.tile(
                    [P, n_chunk],
                    mybir.dt.float32,
                    tag=f"pm{(g0 + mb) % 8}",
                    name=f"pm{(g0 + mb) % 8}",
                )
                for mb in range(m_block)
                if mb != last_mb
            }
            for q in range(4):
                for mb in range(m_block):
                    if mb == last_mb:
                        continue
                    for k in range(KQ):
                        kt = q * KQ + k
                        nc.tensor.matmul(
                            pms[mb][:],
                            x_ap(b, q, k)[:, bass.ts(mb, P)],
                            wm_ap(nt, q, k),
                            start=(kt == 0),
                            stop=(kt == KT - 1),
                        )
                    if q == 3:
                        evac(nt, b * m_block + mb, pms[mb][:])
            if last_mb is not None:
                # final output tile: two half-width accumulation chains so
                # the very last evacuation (copy + DMA) is half-sized —
                # shortens the kernel tail by ~1us
                mb = last_mb
                mt = b * m_block + mb
                h = n_chunk // 2
                qn = n_chunk // 4
                chains = [
                    (pm_pool.tile([P, h], mybir.dt.float32, tag="pm7", name="pmh0"), slice(0, h)),
                    (pm_pool.tile([P, qn], mybir.dt.float32, tag="pm0", name="pmh1"), slice(h, h + qn)),
                    (pm_pool.tile([P, qn], mybir.dt.float32, tag="pm1", name="pmh2"), slice(h + qn, n_chunk)),
                ]
                for hp, csl in chains:
                    for kt in range(KT):
                        q, k = kt // KQ, kt % KQ
                        nc.tensor.matmul(
                            hp[:],
                            x_ap(b, q, k)[:, bass.ts(mb, P)],
                            wm_ap(nt, q, k)[:, csl],
                            start=(kt == 0),
                            stop=(kt == KT - 1),
                        )
                    evac(nt, mt, hp[:], csl)

        # b0 row first (paced by the W stream), then nt-outer over the rest
        for nt in range(NT):
            pair(nt, 0)
        for nt in range(NT):
            for b in range(1, NB):
                pair(nt, b)

    nc.compile()
    return nc


def _prep_host(input_, weight, mask, n_chunk=512, m_block=4):
    bf = ml_dtypes.bfloat16
    in_dim, out_dim = weight.shape[1], weight.shape[0]
    nt = out_dim // n_chunk
    kt = in_dim // P
    rows = input_.shape[0] // N_CORES
    nb = rows // (m_block * P)
    mw = m_block * P
    # W^T as partition-major panels: [NT, P, KT, n_chunk]
    # weight.T is [in, out]; in = k*P + p (k outer)
    wtp = np.ascontiguousarray(
        weight.T.reshape(kt, P, nt, n_chunk).transpose(2, 1, 0, 3)
    ).astype(bf)
    mkp = np.ascontiguousarray(
        mask.T.reshape(kt, P, nt, n_chunk).transpose(2, 1, 0, 3)
    ).astype(np.int8)
    # nibble-pack two adjacent ktiles per byte: bit0 = even kt, bit1 = odd
    mkp = np.ascontiguousarray(mkp[:, :, 0::2, :] | (mkp[:, :, 1::2, :] << 1))
    in_maps = []
    for c in range(N_CORES):
        xc = input_[c * rows : (c + 1) * rows]  # [rows, in]
        # x^T [in, rows] -> [P, NB, KT, mw]
        xp = np.ascontiguousarray(
            xc.T.reshape(kt, P, nb, mw).transpose(1, 2, 0, 3)
        ).astype(bf)
        in_maps.append({"x": xp, "wt": wtp, "mk": mkp})
    return in_maps


_CACHE = {}


def _run(input_, weight, mask, trace=False, **build_kw):
    rows_total, in_dim = input_.shape
    out_dim = weight.shape[0]
    key = (rows_total, in_dim, out_dim, tuple(sorted(build_kw.items())))
    if key not in _CACHE:
        _CACHE[key] = build_nc(
            rows=rows_total // N_CORES, in_dim=in_dim, out_dim=out_dim, **build_kw
        )
    nc = _CACHE[key]
    in_maps = _prep_host(
        input_,
        weight,
        mask,
        build_kw.get("n_chunk", 512),
        build_kw.get("m_block", 4),
    )
    res = run_bass_kernel_spmd(nc, in_maps, core_ids=list(range(N_CORES)), trace=trace)
    out = np.concatenate([res.results[c]["y"] for c in range(N_CORES)], axis=0)
    return out, res


def kernel(input_, weight, mask):
    input_ = np.asarray(input_, dtype=np.float32)
    weight = np.asarray(weight, dtype=np.float32)
    mask = np.asarray(mask)
    out, _ = _run(input_, weight, mask, trace=False)
    return out


# revision 35
# speedup vs baseline: 1.1752x; 1.1752x over previous
"""Masked (expander) linear layer on 8 Trainium2 NeuronCores.

Computes out = x @ (W * M)^T for
  x: [16384, 2048] f32, W: [2048, 2048] f32, M: [2048, 2048] int32 (0/1)

Sharding: pure data-parallel over rows of x. Each of the 8 cores gets 2048
rows of x plus a replicated copy of W and M, computes its [2048, 2048]
output shard entirely locally, and the host concatenates shards. No
collectives.

Device-side design (v6):
 - x and W are repacked to bf16 on the host (the same rounding a
   device-side cast-DMA would apply; 2.1e-3 rel err, far under
   tolerance), the mask to int8. Input HBM traffic per core is 20MB.
   The mask multiply (the module's elementwise FLOPs) runs on DVE; all
   matmul FLOPs run on PE.
 - Host layouts are PARTITION-MAJOR panels (x: [P, NB, KT, 512],
   W/mask: [NT, P, KT, 512]) so every DMA piece is 4-16KB contiguous
   per partition: descriptor generation is cheap and the stream runs at
   near line rate. This is what lets the PE start ~10us in without
   outrunning the stream.
 - Everything is SBUF-resident (wm 64KB/partition, x 64KB/partition,
   bf16); x is loaded exactly once.
 - All input DMAs ride the sync HWDGE ring in exact PE consumption
   order: x block0 (first quarter per single ktile) interleaved with
   n-chunk 0 of mask/W, then n-chunks 1-3, then x blocks 1-3. y
   evacuations ride the scalar ring (ACT copy + ACT-triggered DMA).
 - PE order: pairs (nt, b=0) for nt 0..3 first (paced by the W stream),
   then nt-outer over blocks 1..3 (everything resident by then). Steady
   state is LDWEIGHTS-hidden (bf16 fast-weight-load) and runs at the
   216ns/matmul streaming floor, gapless.
 - A warm-up burst of tiny matmuls on a scratch PSUM bank bridges the
   first DMA latency so the HAM clock-gate is at full rate (2.4 GHz)
   when real matmuls start.
 - PSUM groups rotate over all 8 banks; each group is evacuated right
   after it closes.
"""

from contextlib import ExitStack

import numpy as np
import ml_dtypes

import concourse.bacc as bacc
import concourse.bass as bass
import concourse.mybir as mybir
import concourse.tile as tile
from concourse.bass_utils import run_bass_kernel_spmd

N_CORES = 8
P = 128

FULL_N, FULL_OUT, FULL_IN = 16384, 2048, 2048


def build_nc(
    rows: int = FULL_N // N_CORES,
    in_dim: int = FULL_IN,
    out_dim: int = FULL_OUT,
    n_chunk: int = 512,
    m_block: int = 4,
    warmup_mms: int = 140,
):
    """Per-core Bass module: y[rows, out] = x @ (wt * m), bf16 inputs.

    DRAM layouts (partition-major panels):
      x  [P, NB, KT, m_block*P] bf16
      wt [NT, P, KT, n_chunk]   bf16
      mk [NT, P, KT, n_chunk]   int8
      y  [rows, out_dim]        f32 (row-major)
    """
    assert rows % P == 0 and in_dim % P == 0 and out_dim % n_chunk == 0
    KT = in_dim // P
    MT = rows // P
    NT = out_dim // n_chunk
    assert KT % 4 == 0 and MT % m_block == 0
    KQ = KT // 4
    NB = MT // m_block
    mw = m_block * P  # columns of x per block

    bf16 = mybir.dt.bfloat16

    nc = bacc.Bacc("TRN2", target_bir_lowering=False, debug=False)
    x = nc.dram_tensor("x", [P, NB, KT, mw], bf16, kind="ExternalInput")
    # x block0/quarter0 duplicated in mb-major order: the PE's first
    # psum group needs only one 128KB piece of it
    x00 = nc.dram_tensor("x00", [m_block, P, KQ, P], bf16, kind="ExternalInput")
    wt = nc.dram_tensor("wt", [NT, P, KT, n_chunk], bf16, kind="ExternalInput")
    mk = nc.dram_tensor("mk", [NT, P, KT, n_chunk], mybir.dt.int8, kind="ExternalInput")
    y = nc.dram_tensor("y", [rows, out_dim], mybir.dt.float32, kind="ExternalOutput")

    with ExitStack() as ctx:
        tc = ctx.enter_context(tile.TileContext(nc))
        wm_pool = ctx.enter_context(tc.tile_pool(name="wm", bufs=1))
        xt_pool = ctx.enter_context(tc.tile_pool(name="xt", bufs=1))
        ws_pool = ctx.enter_context(tc.tile_pool(name="ws", bufs=4))
        msk_pool = ctx.enter_context(tc.tile_pool(name="msk", bufs=4))
        yo_pool = ctx.enter_context(tc.tile_pool(name="yo", bufs=3))
        wu_pool = ctx.enter_context(tc.tile_pool(name="wu", bufs=1))
        pm_pool = ctx.enter_context(tc.tile_pool(name="pm", bufs=1, space="PSUM"))

        # Resident masked weight: wm_t[nt][q] of shape [P, KQ, n_chunk]
        # bf16; (nt0, q0) as two 2-ktile halves so the first matmul's W
        # dependency is 0.375MB instead of 1.25MB
        wm_t = [
            [
                wm_pool.tile([P, KQ, n_chunk], bf16, tag=f"wm{nt}_{q}", name=f"wm{nt}_{q}")
                if not (nt == 0 and q == 0)
                else None
                for q in range(4)
            ]
            for nt in range(NT)
        ]
        wm00h = [
            wm_pool.tile([P, 2, n_chunk], bf16, tag=f"wm00h{i}", name=f"wm00h{i}")
            for i in range(2)
        ]
        # Resident x: xt_t[b][q] tiles [P, KQ, mw]; (b0, q0) as m_block
        # mb-column pieces (128KB each) for the fine-grained chase
        xt_t = [
            [
                xt_pool.tile([P, KQ, mw], bf16, tag=f"xt{b}_{q}", name=f"xt{b}_{q}")
                if not (b == 0 and q == 0)
                else None
                for q in range(4)
            ]
            for b in range(NB)
        ]
        xt00mb = [
            xt_pool.tile([P, KQ, P], bf16, tag=f"xmb{mb}", name=f"xmb{mb}")
            for mb in range(m_block)
        ]

        def x_ap(b, q, k, mb):
            if b == 0 and q == 0:
                return xt00mb[mb][:, k, :]
            return xt_t[b][q][:, k, bass.ts(mb, P)]

        def wm_ap(nt, q, k):
            if nt == 0 and q == 0:
                return wm00h[k // 2][:, k % 2, :]
            return wm_t[nt][q][:, k, :]

        # ---- PE warm-up: tiny matmuls on scratch data bridge the first
        # DMA latency and keep the HAM activity window busy so real
        # matmuls start at the full 2.4 GHz clock.
        if warmup_mms:
            wu = wu_pool.tile([P, P], bf16, tag="wu", name="wu")
            nc.vector.memset(wu[:], 0.0)
            pwu = pm_pool.tile([P, 64], mybir.dt.float32, tag="pm7", name="pmwu")
            for i in range(warmup_mms):
                nc.tensor.matmul(pwu[:], wu[:], wu[:, :64], start=True, stop=True)

        def load_x_piece(b, q):
            ksl = slice(q * KQ, (q + 1) * KQ)
            nc.sync.dma_start(out=xt_t[b][q][:], in_=x[:, b, ksl, :])

        def load_w_piece(nt, q):
            ksl = slice(q * KQ, (q + 1) * KQ)
            mtile = msk_pool.tile([P, KQ, n_chunk], mybir.dt.int8, tag="mt")
            nc.sync.dma_start(out=mtile[:], in_=mk[nt, :, ksl, :])
            wstage = ws_pool.tile([P, KQ, n_chunk], bf16, tag="ws")
            nc.sync.dma_start(out=wstage[:], in_=wt[nt, :, ksl, :])
            # masked multiply on DVE (bf16: 2x throughput), one op per piece
            nc.vector.tensor_mul(wm_t[nt][q][:], wstage[:], mtile[:])

        # ---- input stream, in exact PE consumption order. Two HWDGE
        # rings ONLY through x block0 (2MB) — after that the scalar ring
        # is free for y evacuations and the sync ring gets the full
        # ~360GB/s. Fine pieces up front: W/mask 2-ktile halves on sync,
        # x mb-columns on scalar, so the first matmul's dependency is
        # ~0.5MB and the PE chases the stream from ~13.5us.
        for i in range(2):
            hsl = slice(i * 2, (i + 1) * 2)
            mth = msk_pool.tile([P, 2, n_chunk], mybir.dt.int8, tag="mt0")
            nc.sync.dma_start(out=mth[:], in_=mk[0, :, hsl, :])
            wsh = ws_pool.tile([P, 2, n_chunk], bf16, tag="ws0")
            nc.sync.dma_start(out=wsh[:], in_=wt[0, :, hsl, :])
            nc.vector.tensor_mul(wm00h[i][:], wsh[:], mth[:])
        for q in range(1, 4):
            load_w_piece(0, q)
        for nt in range(1, NT):
            for q in range(4):
                load_w_piece(nt, q)
        for b in range(1, NB):
            for q in range(4):
                load_x_piece(b, q)
        # scalar ring: x block0 (mb pieces for q0, quarters for q1-3)
        for mb in range(m_block):
            nc.scalar.dma_start(out=xt00mb[mb][:], in_=x00[mb, :, :, :])
        for q in range(1, 4):
            ksl = slice(q * KQ, (q + 1) * KQ)
            nc.scalar.dma_start(out=xt_t[0][q][:], in_=x[:, 0, ksl, :])

        # ---- PE: pair (nt, b) = m_block PSUM groups of KT matmuls each
        pair_idx = [0]
        n_pairs = NT * NB

        def evac(nt, mt, psum_ap, csl=slice(0, None)):
            cols = (csl.stop or n_chunk) - (csl.start or 0)
            yo = yo_pool.tile([P, n_chunk], mybir.dt.float32, tag="yo")
            nc.scalar.copy(yo[:, 0:cols], psum_ap)
            nc.scalar.dma_start(
                out=y[
                    mt * P : (mt + 1) * P,
                    nt * n_chunk + (csl.start or 0) : nt * n_chunk
                    + (csl.start or 0)
                    + cols,
                ],
                in_=yo[:, 0:cols],
            )

        def pair(nt, b):
            g0 = pair_idx[0] * m_block
            last_pair = pair_idx[0] == n_pairs - 1
            pair_idx[0] += 1
            last_mb = m_block - 1 if last_pair else None
            pms = {
                mb: pm_pool.tile(
                    [P, n_chunk],
                    mybir.dt.float32,
                    tag=f"pm{(g0 + mb) % 8}",
                    name=f"pm{(g0 + mb) % 8}",
                )
                for mb in range(m_block)
                if mb != last_mb
            }
            for q in range(4):
                for mb in range(m_block):
                    if mb == last_mb:
                        continue
                    for k in range(KQ):
                        kt = q * KQ + k
                        nc.tensor.matmul(
                            pms[mb][:],
                            x_ap(b, q, k, mb),
                            wm_ap(nt, q, k),
                            start=(kt == 0),
                            stop=(kt == KT - 1),
                        )
                    if q == 3:
                        evac(nt, b * m_block + mb, pms[mb][:])
            if last_mb is not None:
                # final output tile: two half-width accumulation chains so
                # the very last evacuation (copy + DMA) is half-sized —
                # shortens the kernel tail by ~1us
                mb = last_mb
                mt = b * m_block + mb
                for i, hp in enumerate(
                    [
                        pm_pool.tile([P, n_chunk // 2], mybir.dt.float32, tag="pm7", name="pmh0"),
                        pm_pool.tile([P, n_chunk // 2], mybir.dt.float32, tag="pm0", name="pmh1"),
                    ]
                ):
                    csl = slice(i * (n_chunk // 2), (i + 1) * (n_chunk // 2))
                    for kt in range(KT):
                        q, k = kt // KQ, kt % KQ
                        nc.tensor.matmul(
                            hp[:],
                            x_ap(b, q, k, mb),
                            wm_ap(nt, q, k)[:, csl],
                            start=(kt == 0),
                            stop=(kt == KT - 1),
                        )
                    evac(nt, mt, hp[:], csl)

        # b0 row first (paced by the W stream), then nt-outer over the rest
        for nt in range(NT):
            pair(nt, 0)
        for nt in range(NT):
            for b in range(1, NB):
                pair(nt, b)

    nc.compile()
    return nc


def _prep_host(input_, weight, mask, n_chunk=512, m_block=4):
    bf = ml_dtypes.bfloat16
    in_dim, out_dim = weight.shape[1], weight.shape[0]
    nt = out_dim // n_chunk
    kt = in_dim // P
    rows = input_.shape[0] // N_CORES
    nb = rows // (m_block * P)
    mw = m_block * P
    # W^T as partition-major panels: [NT, P, KT, n_chunk]
    # weight.T is [in, out]; in = k*P + p (k outer)
    wtp = np.ascontiguousarray(
        weight.T.reshape(kt, P, nt, n_chunk).transpose(2, 1, 0, 3)
    ).astype(bf)
    mkp = np.ascontiguousarray(
        mask.T.reshape(kt, P, nt, n_chunk).transpose(2, 1, 0, 3)
    ).astype(np.int8)
    in_maps = []
    for c in range(N_CORES):
        xc = input_[c * rows : (c + 1) * rows]  # [rows, in]
        # x^T [in, rows] -> [P, NB, KT, mw]
        xp = np.ascontiguousarray(
            xc.T.reshape(kt, P, nb, mw).transpose(1, 2, 0, 3)
        ).astype(bf)
        # block0/quarter0 duplicated mb-major: [m_block, P, KQ, P]
        kq = kt // 4
        x00 = np.ascontiguousarray(
            xp[:, 0, 0:kq, :].reshape(P, kq, m_block, P).transpose(2, 0, 1, 3)
        )
        in_maps.append({"x": xp, "x00": x00, "wt": wtp, "mk": mkp})
    return in_maps


_CACHE = {}


def _run(input_, weight, mask, trace=False, **build_kw):
    rows_total, in_dim = input_.shape
    out_dim = weight.shape[0]
    key = (rows_total, in_dim, out_dim, tuple(sorted(build_kw.items())))
    if key not in _CACHE:
        _CACHE[key] = build_nc(
            rows=rows_total // N_CORES, in_dim=in_dim, out_dim=out_dim, **build_kw
        )
    nc = _CACHE[key]
    in_maps = _prep_host(
        input_,
        weight,
        mask,
        build_kw.get("n_chunk", 512),
        build_kw.get("m_block", 4),
    )
    res = run_bass_kernel_spmd(nc, in_maps, core_ids=list(range(N_CORES)), trace=trace)
    out = np.concatenate([res.results[c]["y"] for c in range(N_CORES)], axis=0)
    return out, res


def kernel(input_, weight, mask):
    input_ = np.asarray(input_, dtype=np.float32)
    weight = np.asarray(weight, dtype=np.float32)
    mask = np.asarray(mask)
    out, _ = _run(input_, weight, mask, trace=False)
    return out
